# revision 23
# baseline (speedup 1.0000x reference)
"""Trainium2 Bass kernel: 2-layer GraphSAGE (mean aggregation), 8-core SPMD.

nn_BiGNN: out = sage2(relu(sage1(x)));  sage(x) = mean_{j->i}(x_j) @ W_l + b_l + x @ W_r
N=50000 nodes, E=800000 edges, d=128, f32 inputs / f32 output.

Strategy (one NeuronCore owns 6250 destination nodes):
  - host: partition edges by destination block, sort by dst, pad per
    128-dst subwindow, equalize batch counts across cores (SPMD).
    Gather idxs are SIGNED int16 against a table view based mid-table
    (the SWDGE ucode multiplies the stride by the signed index), so a
    single phase covers up to 65536 rows.  Within each 128-lane batch a
    nonnegative idx is kept in the last lane (the ucode trims trailing
    negatives).
  - device: SWDGE dma_gather of bf16 source rows round-robined over 4
    SWDGE queues (each queue runs on its own Q7 core pair -> 4x parallel
    descriptor generation); one-hot segment matrices on DVE; TensorE
    matmul msg^T @ seg accumulated per 512-node PSUM window = transposed
    mean-aggregation; 1/deg folded into the PSUM evacuation; weight
    matmuls + bias + relu.
  - between layers, h is exchanged in TWO AllGather chunks (separate
    Shared tiles: the scheduler allows one writer per Shared tile):
    chunk A = local rows [0, 4096) fired after window 7 and hidden under
    the remaining layer-1 compute; chunk B = rows [4096, 6250) at the
    end.  Layer-2 gathers run in two phases (one per chunk table); each
    phase's table view overlaps its chunk's written range so the
    dependency is tracked.
"""

import os
import sys
import types

for _p in ("/opt/trn_rl_repo", "/root/.axon_site/_ro/trn_rl_repo",
           "/root/.axon_site"):
    if os.path.isdir(_p) and _p not in sys.path:
        sys.path.insert(0, _p)


def _install_ntff_hook():
    """Provide antenv.axon_hooks (missing in this image) so trace=True can
    capture NTFF profiles through libaxon_pjrt.so."""
    if "antenv.axon_hooks" in sys.modules:
        return
    store = [None]
    mod = types.ModuleType("antenv.axon_hooks")
    mod.set_axon_ntff_profile_hook = lambda h: store.__setitem__(0, h)
    mod.get_axon_ntff_profile_hook = lambda: store[0]
    sys.modules["antenv.axon_hooks"] = mod
    try:
        import antenv
        antenv.axon_hooks = mod
        from trn_agent_boot.trn_boot import _ntff_profile_via_ctypes
        so = "/opt/axon/libaxon_pjrt.so"
        if os.path.exists(so):
            mod.set_axon_ntff_profile_hook(_ntff_profile_via_ctypes(so))
    except Exception:
        pass


_install_ntff_hook()


import numpy as np
import ml_dtypes

import concourse.bass as bass
import concourse.bacc as bacc
import concourse.mybir as mybir
import concourse.tile as tile
from concourse.library_config import mlp as mlp_library

P = 128
D = 128
GMAX = 8  # max batches (1024 idxs) per dma_gather instruction (ring limit)
NQ = 4  # SWDGE queues (Q7 core pairs)
F32 = mybir.dt.float32
BF16 = mybir.dt.bfloat16
I16 = mybir.dt.int16

NPCA = 4096  # local rows in AllGather chunk A (must be multiple of 512)


def wrap_idx16(arr):
    """[n] int array -> [128, n//16] int16 SWDGE layout (16-partition wrap,
    replicated for the 8 Q7 cores)."""
    n = arr.shape[0]
    assert n % 16 == 0
    w = np.asarray(arr, dtype=np.int16).reshape(n // 16, 16).T  # [16, n/16]
    return np.tile(w, (8, 1))  # [128, n/16]


# ----------------------------------------------------------------- host prep
def host_prep(edge_index, n_nodes, ncores, win=512):
    """Partition/sort edges per core and destination subwindow; build the
    per-layer gather streams.

    Layer 1: single phase, table = x based at row 32768.
    Layer 2: two phases by source chunk (A: local row < NPCA, B: rest),
    tables = h_fullA [ncores*NPCA] / h_fullB [ncores*(npc-NPCA)] with
    core-major positions.
    """
    npc = n_nodes // ncores
    nsub = (npc + P - 1) // P
    nwin = (npc + win - 1) // win
    spw = win // P
    src_a = np.asarray(edge_index[0], dtype=np.int64)
    dst_a = np.asarray(edge_index[1], dtype=np.int64)

    edges = [[None] * nsub for _ in range(ncores)]
    invcnt = np.zeros((ncores, 1, npc), dtype=np.float32)
    for c in range(ncores):
        lo_n = c * npc
        m = (dst_a >= lo_n) & (dst_a < lo_n + npc)
        s, d = src_a[m], dst_a[m] - lo_n
        invcnt[c, 0] = 1.0 / np.maximum(np.bincount(d, minlength=npc), 1.0)
        order = np.argsort(d, kind="stable")
        s, d = s[order], d[order]
        sub = d // P
        for t in range(nsub):
            ms = sub == t
            edges[c][t] = (s[ms], d[ms])

    npcb = npc - NPCA
    sizeA, sizeB = ncores * NPCA, ncores * npcb

    # positions in the phase tables; idx = pos - base (signed int16)
    def pos_phase0(s):  # layer 1: identity, base 32768
        return s - 32768

    def posA(s):  # chunk A: core-major [ncores, NPCA], base sizeA // 2
        c, r = s // npc, s % npc
        return c * NPCA + r - sizeA // 2

    def posB(s):  # chunk B: core-major [ncores, npcb], base sizeB // 2
        c, r = s // npc, s % npc
        return c * npcb + (r - NPCA) - sizeB // 2

    def build_stream(nphases, phase_of, idx_of):
        """-> dict(blocks, ncols, idx16, slots) for one layer's gathers."""
        # per (core, sub, phase) edge lists
        by = [[[None] * nphases for _ in range(nsub)] for _ in range(ncores)]
        for c in range(ncores):
            for t in range(nsub):
                s, d = edges[c][t]
                ph = phase_of(s)
                for p_ in range(nphases):
                    mp = ph == p_
                    by[c][t][p_] = (s[mp], d[mp])
        nb = np.zeros((nsub, nphases), dtype=np.int64)
        for c in range(ncores):
            for t in range(nsub):
                for p_ in range(nphases):
                    n = len(by[c][t][p_][0])
                    nb[t, p_] = max(nb[t, p_], (n + P - 1) // P)
        # every (window, phase) needs >=1 batch so each PSUM group gets its
        # start/stop flags and every window emits output in the final phase;
        # with random edges this never binds
        nb = np.maximum(nb, 1)

        # phase-major block order: all phase-0 windows, then phase-1 — the
        # phase-1 gathers (layer 2: chunk-B table) only start after every
        # phase-0 block, letting the B AllGather overlap the whole A pass
        blocks = []
        ncols = 0
        for p_ in range(nphases):
            for w in range(nwin):
                subs = range(w * spw, min((w + 1) * spw, nsub))
                bl = [(t, int(nb[t, p_])) for t in subs]
                nbl = sum(x[1] for x in bl)
                if nbl:
                    blocks.append(dict(w=w, ph=p_, col0=ncols, nb=nbl, subs=bl))
                    ncols += nbl

        idx16 = np.zeros((ncores, P, ncols * 8), dtype=np.int16)
        slots = np.zeros((ncores, P, ncols), dtype=ml_dtypes.bfloat16)
        for c in range(ncores):
            for blk in blocks:
                col = blk["col0"]
                for t, nbt in blk["subs"]:
                    s, d = by[c][t][blk["ph"]]
                    n = len(s)
                    npad = nbt * P
                    si = np.zeros(npad, dtype=np.int64)  # pad -> idx 0 (base row)
                    if n:
                        si[:n] = idx_of[blk["ph"]](s)
                    sl = np.full(npad, -1.0, dtype=np.float32)
                    sl[:n] = (d % P).astype(np.float32)
                    sb = si.reshape(nbt, P)
                    slb = sl.reshape(nbt, P)
                    order = np.argsort(sb >= 0, axis=1, kind="stable")
                    sb = np.take_along_axis(sb, order, axis=1)
                    slb = np.take_along_axis(slb, order, axis=1)
                    for b in range(nbt):
                        slots[c, :, col + b] = slb[b].astype(ml_dtypes.bfloat16)
                    idx16[c, :, col * 8:(col + nbt) * 8] = \
                        wrap_idx16(sb.reshape(-1))
                    col += nbt
        assert np.all(np.abs(idx16) <= 32767)
        return dict(blocks=blocks, ncols=ncols, idx16=idx16, slots=slots)

    s0 = build_stream(1, lambda s: np.zeros(len(s), np.int64),
                      [pos_phase0])
    s1 = build_stream(2, lambda s: ((s % npc) >= NPCA).astype(np.int64),
                      [posA, posB])

    return dict(npc=npc, nsub=nsub, nwin=nwin, win=win, npcb=npcb,
                sizeA=sizeA, sizeB=sizeB, streams=[s0, s1], invcnt=invcnt)


# -------------------------------------------------------------- kernel build
def build_kernel(n_nodes, ncores, prep, nb_onehot=8):
    npc, nwin, win = prep["npc"], prep["nwin"], prep["win"]
    npcb, sizeA, sizeB = prep["npcb"], prep["sizeA"], prep["sizeB"]
    streams = prep["streams"]
    spw = win // P

    nc = bacc.Bacc(None, num_swdge_queues=NQ)

    xtab = nc.declare_dram_parameter("xtab", [n_nodes, D], BF16, isOutput=False)
    xT_d = nc.declare_dram_parameter("xT", [D, npc], F32, isOutput=False)
    nc0, nc1 = streams[0]["ncols"], streams[1]["ncols"]
    idx0_d = nc.declare_dram_parameter("idx16_0", [P, nc0 * 8], I16, isOutput=False)
    idx1_d = nc.declare_dram_parameter("idx16_1", [P, nc1 * 8], I16, isOutput=False)
    slots0_d = nc.declare_dram_parameter("slots_0", [P, nc0], BF16, isOutput=False)
    slots1_d = nc.declare_dram_parameter("slots_1", [P, nc1], BF16, isOutput=False)
    invcnt_d = nc.declare_dram_parameter("invcnt", [P, npc], F32, isOutput=False)
    W1l_d = nc.declare_dram_parameter("W1l", [D, D], F32, isOutput=False)
    W1r_d = nc.declare_dram_parameter("W1r", [D, D], F32, isOutput=False)
    W2l_d = nc.declare_dram_parameter("W2l", [D, D], F32, isOutput=False)
    W2r_d = nc.declare_dram_parameter("W2r", [D, D], F32, isOutput=False)
    b1_d = nc.declare_dram_parameter("b1", [D, 1], F32, isOutput=False)
    b2row_d = nc.declare_dram_parameter("b2row", [P, D], F32, isOutput=False)
    iota_d = nc.declare_dram_parameter("iota", [P, P], BF16, isOutput=False)
    ident_d = nc.declare_dram_parameter("ident", [P, P], F32, isOutput=False)
    out_d = nc.declare_dram_parameter("out", [npc, D], F32, isOutput=True)

    from contextlib import ExitStack
    with tile.TileContext(nc) as tc, ExitStack() as es:
        dram = es.enter_context(tc.tile_pool(name="dram", bufs=1, space="DRAM"))
        h_local = dram.tile([npc, D], BF16, tag="hloc")
        h_fullA = dram.tile([sizeA, D], BF16, tag="hfA", addr_space="Shared")
        h_fullB = dram.tile([sizeB, D], BF16, tag="hfB", addr_space="Shared")

        const = es.enter_context(tc.tile_pool(name="const", bufs=1))
        sb = es.enter_context(tc.tile_pool(name="sb", bufs=1))
        msgp = es.enter_context(tc.tile_pool(name="msgp", bufs=6))
        segp = es.enter_context(tc.tile_pool(name="segp", bufs=4))
        aggp = es.enter_context(tc.tile_pool(name="aggp", bufs=2))
        rowp = es.enter_context(tc.tile_pool(name="rowp", bufs=3))
        psA = es.enter_context(tc.tile_pool(name="psA", bufs=2, space="PSUM"))
        psB = es.enter_context(tc.tile_pool(name="psB", bufs=2, space="PSUM"))
        psT = es.enter_context(tc.tile_pool(name="psT", bufs=2, space="PSUM"))

        nc.gpsimd.load_library(mlp_library)

        idx0_sb = const.tile([P, nc0 * 8], I16, tag="idx0")
        idx1_sb = const.tile([P, nc1 * 8], I16, tag="idx1")
        slots0_sb = const.tile([P, nc0], BF16, tag="slots0")
        slots1_sb = const.tile([P, nc1], BF16, tag="slots1")
        invcnt_sb = const.tile([P, npc], F32, tag="invcnt")
        iota_sb = const.tile([P, P], BF16, tag="iota")
        ident_sb = const.tile([P, P], F32, tag="ident")
        W1l_sb = const.tile([D, D], F32, tag="W1l")
        W1r_sb = const.tile([D, D], F32, tag="W1r")
        W2l_sb = const.tile([D, D], F32, tag="W2l")
        W2r_sb = const.tile([D, D], F32, tag="W2r")
        b1_sb = const.tile([D, 1], F32, tag="b1")
        b2row_sb = const.tile([P, D], F32, tag="b2row")
        xT_sb = sb.tile([D, npc], F32, tag="xT")
        hT_sb = sb.tile([D, npc], F32, tag="hT")

        # load order matters: the first gather only needs idx0; the first
        # window's matmuls need slots0/iota/weights/xT; invcnt at first
        # evacuation; layer-2 tables last.
        for t, dd in [(idx0_sb, idx0_d), (slots0_sb, slots0_d),
                      (iota_sb, iota_d), (b1_sb, b1_d),
                      (W1l_sb, W1l_d), (W1r_sb, W1r_d), (xT_sb, xT_d),
                      (invcnt_sb, invcnt_d), (ident_sb, ident_d),
                      (b2row_sb, b2row_d), (W2l_sb, W2l_d), (W2r_sb, W2r_d),
                      (idx1_sb, idx1_d), (slots1_sb, slots1_d)]:
            nc.sync.dma_start(out=t[:], in_=dd[:])

        gq = [0]

        def emit_layer(layer, tabs, idx_sb, slots_sb, stream):
            ncols, blocks = stream["ncols"], stream["blocks"]
            nphases = max(blk["ph"] for blk in blocks) + 1
            ngrp = (ncols + nb_onehot - 1) // nb_onehot
            segs = []
            for g in range(ngrp):
                nbg = min(nb_onehot, ncols - g * nb_onehot)
                seg = segp.tile([P, nb_onehot, P], BF16, tag="seg",
                                name=f"seg{layer}_{g}")
                g0 = g * nb_onehot
                nc.vector.tensor_tensor(
                    out=seg[:, :nbg, :],
                    in0=iota_sb[:, None, :].to_broadcast([P, nbg, P]),
                    in1=slots_sb[:, g0:g0 + nbg, None].to_broadcast([P, nbg, P]),
                    op=mybir.AluOpType.is_equal,
                )
                segs.append(seg)

            # phase-partial aggregation buffer (phases accumulate into SBUF;
            # the last phase combines and emits the window's outputs)
            aggpart = None
            if nphases > 1:
                aggpart = sb.tile([P, npc], F32, tag=f"aggpart{layer}",
                                  name=f"aggpart{layer}")

            for blk in blocks:
                w, ph = blk["w"], blk["ph"]
                n0 = w * win
                wn = min(win, npc - n0)
                nsw = (wn + P - 1) // P
                agg_ps = psA.tile([P, win], F32, tag="aggT",
                                  name=f"agg{layer}_{w}_{ph}")

                # each block is its own PSUM accumulation group (the start
                # flag arms zero-on-first-write for the whole 2KB bank)
                sub_of_b = {}
                col = blk["col0"]
                for t, nbt in blk["subs"]:
                    for bi in range(nbt):
                        sub_of_b[col + bi] = t
                    col += nbt
                blk_first_b = blk["col0"]
                blk_last_b = blk["col0"] + blk["nb"] - 1

                # split the block into near-equal instruction sizes: uniform
                # desc-gen times keep the 4 SWDGE queue pairs in lockstep
                # (a small tail instruction breaks the round-robin rhythm)
                nin = (blk["nb"] + GMAX - 1) // GMAX
                base, extra = divmod(blk["nb"], nin)
                tab = tabs[ph]
                c0 = 0
                for ii in range(nin):
                    cn = base + (1 if ii < extra else 0)
                    msg = msgp.tile([P, GMAX, D], BF16, tag="msg",
                                    name=f"msg{layer}_{w}_{ph}_{ii}")
                    nidx = cn * P
                    b0 = blk["col0"] + c0
                    qn = gq[0]
                    gq[0] = (qn + 1) % NQ
                    nc.gpsimd.dma_gather(
                        out_ap=msg[:, :cn, :],
                        in_ap=tab,
                        idxs_ap=idx_sb[:, b0 * 8:(b0 + cn) * 8],
                        num_idxs=nidx,
                        num_idxs_reg=nidx,
                        elem_size=D,
                        queue_num=qn,
                    )
                    for bi in range(cn):
                        b = b0 + bi
                        t = sub_of_b[b]
                        j = t - w * spw
                        nsl = min(P, npc - t * P)
                        nc.tensor.matmul(
                            out=agg_ps[:, j * P:j * P + nsl],
                            lhsT=msg[:, bi, :],
                            rhs=segs[b // nb_onehot][:, b % nb_onehot, :nsl],
                            start=(b == blk_first_b), stop=(b == blk_last_b),
                        )
                    c0 += cn

                if ph < nphases - 1:
                    # partial phase: stash the window's raw sums in SBUF
                    nc.vector.tensor_copy(out=aggpart[:, n0:n0 + wn],
                                          in_=agg_ps[:, :wn])
                    continue

                aggTs = aggp.tile([P, win], F32, tag="aggTs",
                                  name=f"aggTs{layer}_{w}")
                if nphases > 1:
                    nc.vector.tensor_tensor(
                        out=aggTs[:, :wn], in0=agg_ps[:, :wn],
                        in1=aggpart[:, n0:n0 + wn], op=mybir.AluOpType.add)
                    nc.vector.tensor_tensor(
                        out=aggTs[:, :wn], in0=aggTs[:, :wn],
                        in1=invcnt_sb[:, n0:n0 + wn], op=mybir.AluOpType.mult)
                else:
                    nc.vector.tensor_tensor(
                        out=aggTs[:, :wn], in0=agg_ps[:, :wn],
                        in1=invcnt_sb[:, n0:n0 + wn], op=mybir.AluOpType.mult)

                if layer == 0:
                    ab_ps = psB.tile([P, win], F32, tag="AB", name=f"ab{w}")
                    nc.tensor.matmul(out=ab_ps[:, :wn], lhsT=W1l_sb[:],
                                     rhs=aggTs[:, :wn], start=True, stop=False)
                    nc.tensor.matmul(out=ab_ps[:, :wn], lhsT=W1r_sb[:],
                                     rhs=xT_sb[:, n0:n0 + wn], start=False, stop=True)
                    nc.scalar.activation(
                        out=hT_sb[:, n0:n0 + wn], in_=ab_ps[:, :wn],
                        func=mybir.ActivationFunctionType.Relu,
                        bias=b1_sb[:, 0:1], scale=1.0)
                    for j in range(nsw):
                        r0 = n0 + j * P
                        ns = min(P, npc - r0)
                        tr_ps = psT.tile([P, P], F32, tag="tr", name=f"tr{w}_{j}")
                        nc.tensor.transpose(out=tr_ps[:ns, :],
                                            in_=hT_sb[:, r0:r0 + ns],
                                            identity=ident_sb[:])
                        hrow = rowp.tile([P, D], BF16, tag="hrow",
                                         name=f"hrow{w}_{j}")
                        nc.vector.tensor_copy(out=hrow[:ns, :], in_=tr_ps[:ns, :])
                        nc.sync.dma_start(out=h_local[r0:r0 + ns, :],
                                          in_=hrow[:ns, :])
                    if n0 + wn == NPCA:
                        # chunk A complete: AllGather it now; the transfer
                        # hides under the remaining layer-1 windows.  high
                        # priority pins the trigger right after its deps so
                        # the scheduler cannot push it towards its consumer.
                        with tc.high_priority():
                            nc.gpsimd.collective_compute(
                                "AllGather", mybir.AluOpType.bypass,
                                replica_groups=[list(range(ncores))],
                                ins=[h_local[0:NPCA, :]],
                                outs=[h_fullA[:]])
                    elif n0 + wn == npc:
                        with tc.high_priority():
                            nc.gpsimd.collective_compute(
                                "AllGather", mybir.AluOpType.bypass,
                                replica_groups=[list(range(ncores))],
                                ins=[h_local[NPCA:npc, :]],
                                outs=[h_fullB[:]])
                else:
                    for j in range(nsw):
                        r0 = n0 + j * P
                        ns = min(P, npc - r0)
                        o_ps = psT.tile([P, P], F32, tag="tr", name=f"ops{w}_{j}")
                        nc.tensor.matmul(out=o_ps[:ns, :],
                                         lhsT=aggTs[:, j * P:j * P + ns],
                                         rhs=W2l_sb[:], start=True, stop=False)
                        nc.tensor.matmul(out=o_ps[:ns, :],
                                         lhsT=hT_sb[:, r0:r0 + ns],
                                         rhs=W2r_sb[:], start=False, stop=True)
                        orow = rowp.tile([P, D], F32, tag="orow",
                                         name=f"orow{w}_{j}")
                        nc.vector.tensor_tensor(
                            out=orow[:ns, :], in0=o_ps[:ns, :],
                            in1=b2row_sb[:ns, :], op=mybir.AluOpType.add)
                        nc.sync.dma_start(out=out_d[r0:r0 + ns, :],
                                          in_=orow[:ns, :])

        # phase tables: views based mid-table; signed idx covers the rest.
        # each view overlaps its chunk's written range, so the AllGather ->
        # gather dependency is tracked.
        emit_layer(0, [xtab[32768:n_nodes, :]], idx0_sb, slots0_sb, streams[0])
        emit_layer(1, [h_fullA[sizeA // 2:sizeA, :],
                       h_fullB[sizeB // 2:sizeB, :]],
                   idx1_sb, slots1_sb, streams[1])

    nc.finalize()
    return nc


# ---------------------------------------------------------------- in_maps
def make_in_maps(x, edge_index, W1_l, b1_l, W1_r, W2_l, b2_l, W2_r,
                 n_nodes, ncores, win=512):
    prep = host_prep(edge_index, n_nodes, ncores, win=win)
    npc = prep["npc"]
    x = np.asarray(x, dtype=np.float32)
    xtab = x.astype(ml_dtypes.bfloat16)
    xT = np.ascontiguousarray(x.T)
    iota = np.tile(np.arange(P, dtype=np.float32)[None, :], (P, 1)).astype(
        ml_dtypes.bfloat16)
    ident = np.eye(P, dtype=np.float32)
    common = dict(
        xtab=xtab,
        W1l=np.asarray(W1_l, np.float32), W1r=np.asarray(W1_r, np.float32),
        W2l=np.asarray(W2_l, np.float32), W2r=np.asarray(W2_r, np.float32),
        b1=np.asarray(b1_l, np.float32).reshape(D, 1),
        b2row=np.tile(np.asarray(b2_l, np.float32).reshape(1, D), (P, 1)),
        iota=iota, ident=ident,
    )
    s0, s1 = prep["streams"]
    in_maps = []
    for c in range(ncores):
        in_maps.append(dict(
            common,
            xT=np.ascontiguousarray(xT[:, c * npc:(c + 1) * npc]),
            idx16_0=s0["idx16"][c], idx16_1=s1["idx16"][c],
            slots_0=s0["slots"][c], slots_1=s1["slots"][c],
            invcnt=np.tile(prep["invcnt"][c], (P, 1)),
        ))
    return prep, in_maps


# ------------------------------------------------------------------ kernel()
N_NODES = 50000
NCORES = 8

_cache = {}
last_result = None  # BassKernelResults of the most recent run (for test.py)


def kernel(x, edge_index, W1_l, b1_l, W1_r, W2_l, b2_l, W2_r,
           trace=False, trace_kwargs=None):
    """Full inputs in, full output out. Shards across 8 NeuronCores."""
    global last_result
    from concourse.bass_utils import run_bass_kernel_spmd

    x = np.asarray(x)
    edge_index = np.asarray(edge_index)
    n_nodes = x.shape[0]
    assert n_nodes % NCORES == 0

    prep, in_maps = make_in_maps(x, edge_index, W1_l, b1_l, W1_r,
                                 W2_l, b2_l, W2_r, n_nodes, NCORES)
    key = (n_nodes,
           tuple(blk["nb"] for s in prep["streams"] for blk in s["blocks"]))
    if key not in _cache:
        _cache[key] = build_kernel(n_nodes, NCORES, prep)
    nc = _cache[key]

    res = run_bass_kernel_spmd(nc, in_maps, list(range(NCORES)),
                               trace=trace, **(trace_kwargs or {}))
    last_result = res
    out = np.concatenate([res.results[c]["out"] for c in range(NCORES)],
                         axis=0)
    return out.astype(np.float32)


# revision 24
# speedup vs baseline: 1.0938x; 1.0938x over previous
"""Trainium2 Bass kernel: 2-layer GraphSAGE (mean aggregation), 8-core SPMD.

nn_BiGNN: out = sage2(relu(sage1(x)));  sage(x) = mean_{j->i}(x_j) @ W_l + b_l + x @ W_r
N=50000 nodes, E=800000 edges, d=128, f32 inputs / f32 output.

Strategy (one NeuronCore owns 6250 destination nodes):
  - host: partition edges by destination block, sort by dst, pad per
    128-dst subwindow, equalize batch counts across cores (SPMD).
    Gather idxs are SIGNED int16 against a table view based mid-table
    (the SWDGE ucode multiplies the stride by the signed index), so a
    single phase covers up to 65536 rows.  Within each 128-lane batch a
    nonnegative idx is kept in the last lane (the ucode trims trailing
    negatives).
  - device: SWDGE dma_gather of bf16 source rows round-robined over 4
    SWDGE queues (each queue runs on its own Q7 core pair -> 4x parallel
    descriptor generation); one-hot segment matrices on DVE; TensorE
    matmul msg^T @ seg accumulated per 512-node PSUM window = transposed
    mean-aggregation; 1/deg folded into the PSUM evacuation; weight
    matmuls + bias + relu.
  - between layers, h is exchanged in TWO AllGather chunks (separate
    Shared tiles: the scheduler allows one writer per Shared tile):
    chunk A = local rows [0, 4096) fired after window 7 and hidden under
    the remaining layer-1 compute; chunk B = rows [4096, 6250) at the
    end.  Layer-2 gathers run in two phases (one per chunk table); each
    phase's table view overlaps its chunk's written range so the
    dependency is tracked.
"""

import os
import sys
import types

for _p in ("/opt/trn_rl_repo", "/root/.axon_site/_ro/trn_rl_repo",
           "/root/.axon_site"):
    if os.path.isdir(_p) and _p not in sys.path:
        sys.path.insert(0, _p)


def _install_ntff_hook():
    """Provide antenv.axon_hooks (missing in this image) so trace=True can
    capture NTFF profiles through libaxon_pjrt.so."""
    if "antenv.axon_hooks" in sys.modules:
        return
    store = [None]
    mod = types.ModuleType("antenv.axon_hooks")
    mod.set_axon_ntff_profile_hook = lambda h: store.__setitem__(0, h)
    mod.get_axon_ntff_profile_hook = lambda: store[0]
    sys.modules["antenv.axon_hooks"] = mod
    try:
        import antenv
        antenv.axon_hooks = mod
        from trn_agent_boot.trn_boot import _ntff_profile_via_ctypes
        so = "/opt/axon/libaxon_pjrt.so"
        if os.path.exists(so):
            mod.set_axon_ntff_profile_hook(_ntff_profile_via_ctypes(so))
    except Exception:
        pass


_install_ntff_hook()


import numpy as np
import ml_dtypes

import concourse.bass as bass
import concourse.bacc as bacc
import concourse.mybir as mybir
import concourse.tile as tile
from concourse.library_config import mlp as mlp_library

P = 128
D = 128
GMAX = 8  # max batches (1024 idxs) per dma_gather instruction (ring limit)
NQ = 4  # SWDGE queues (Q7 core pairs)
F32 = mybir.dt.float32
BF16 = mybir.dt.bfloat16
I16 = mybir.dt.int16

NPCA = 4096  # local rows in AllGather chunk A (must be multiple of 512)


def wrap_idx16(arr):
    """[n] int array -> [128, n//16] int16 SWDGE layout (16-partition wrap,
    replicated for the 8 Q7 cores)."""
    n = arr.shape[0]
    assert n % 16 == 0
    w = np.asarray(arr, dtype=np.int16).reshape(n // 16, 16).T  # [16, n/16]
    return np.tile(w, (8, 1))  # [128, n/16]


# ----------------------------------------------------------------- host prep
def host_prep(edge_index, n_nodes, ncores, win=512):
    """Partition/sort edges per core and destination subwindow; build the
    per-layer gather streams.

    Layer 1: single phase, table = x based at row 32768.
    Layer 2: two phases by source chunk (A: local row < NPCA, B: rest),
    tables = h_fullA [ncores*NPCA] / h_fullB [ncores*(npc-NPCA)] with
    core-major positions.
    """
    npc = n_nodes // ncores
    nsub = (npc + P - 1) // P
    nwin = (npc + win - 1) // win
    spw = win // P
    src_a = np.asarray(edge_index[0], dtype=np.int64)
    dst_a = np.asarray(edge_index[1], dtype=np.int64)

    edges = [[None] * nsub for _ in range(ncores)]
    invcnt = np.zeros((ncores, 1, npc), dtype=np.float32)
    for c in range(ncores):
        lo_n = c * npc
        m = (dst_a >= lo_n) & (dst_a < lo_n + npc)
        s, d = src_a[m], dst_a[m] - lo_n
        invcnt[c, 0] = 1.0 / np.maximum(np.bincount(d, minlength=npc), 1.0)
        order = np.argsort(d, kind="stable")
        s, d = s[order], d[order]
        sub = d // P
        for t in range(nsub):
            ms = sub == t
            edges[c][t] = (s[ms], d[ms])

    npcb = npc - NPCA
    sizeA, sizeB = ncores * NPCA, ncores * npcb

    # positions in the phase tables; idx = pos - base (signed int16)
    def pos_phase0(s):  # layer 1: identity, base 32768
        return s - 32768

    def posA(s):  # chunk A: core-major [ncores, NPCA], base sizeA // 2
        c, r = s // npc, s % npc
        return c * NPCA + r - sizeA // 2

    def posB(s):  # chunk B: core-major [ncores, npcb], base sizeB // 2
        c, r = s // npc, s % npc
        return c * npcb + (r - NPCA) - sizeB // 2

    def build_stream(nphases, phase_of, idx_of):
        """-> dict(blocks, ncols, idx16, slots) for one layer's gathers."""
        # per (core, sub, phase) edge lists
        by = [[[None] * nphases for _ in range(nsub)] for _ in range(ncores)]
        for c in range(ncores):
            for t in range(nsub):
                s, d = edges[c][t]
                ph = phase_of(s)
                for p_ in range(nphases):
                    mp = ph == p_
                    by[c][t][p_] = (s[mp], d[mp])
        nb = np.zeros((nsub, nphases), dtype=np.int64)
        for c in range(ncores):
            for t in range(nsub):
                for p_ in range(nphases):
                    n = len(by[c][t][p_][0])
                    nb[t, p_] = max(nb[t, p_], (n + P - 1) // P)
        # every (window, phase) needs >=1 batch so each PSUM group gets its
        # start/stop flags and every window emits output in the final phase;
        # with random edges this never binds
        nb = np.maximum(nb, 1)

        # phase-major block order: all phase-0 windows, then phase-1 — the
        # phase-1 gathers (layer 2: chunk-B table) only start after every
        # phase-0 block, letting the B AllGather overlap the whole A pass
        blocks = []
        ncols = 0
        for p_ in range(nphases):
            for w in range(nwin):
                subs = range(w * spw, min((w + 1) * spw, nsub))
                bl = [(t, int(nb[t, p_])) for t in subs]
                nbl = sum(x[1] for x in bl)
                if nbl:
                    blocks.append(dict(w=w, ph=p_, col0=ncols, nb=nbl, subs=bl))
                    ncols += nbl

        idx16 = np.zeros((ncores, P, ncols * 8), dtype=np.int16)
        slots = np.zeros((ncores, P, ncols), dtype=ml_dtypes.bfloat16)
        for c in range(ncores):
            for blk in blocks:
                col = blk["col0"]
                for t, nbt in blk["subs"]:
                    s, d = by[c][t][blk["ph"]]
                    n = len(s)
                    npad = nbt * P
                    si = np.zeros(npad, dtype=np.int64)  # pad -> idx 0 (base row)
                    if n:
                        si[:n] = idx_of[blk["ph"]](s)
                    sl = np.full(npad, -1.0, dtype=np.float32)
                    sl[:n] = (d % P).astype(np.float32)
                    sb = si.reshape(nbt, P)
                    slb = sl.reshape(nbt, P)
                    order = np.argsort(sb >= 0, axis=1, kind="stable")
                    sb = np.take_along_axis(sb, order, axis=1)
                    slb = np.take_along_axis(slb, order, axis=1)
                    for b in range(nbt):
                        slots[c, :, col + b] = slb[b].astype(ml_dtypes.bfloat16)
                    idx16[c, :, col * 8:(col + nbt) * 8] = \
                        wrap_idx16(sb.reshape(-1))
                    col += nbt
        assert np.all(np.abs(idx16) <= 32767)
        return dict(blocks=blocks, ncols=ncols, idx16=idx16, slots=slots)

    s0 = build_stream(1, lambda s: np.zeros(len(s), np.int64),
                      [pos_phase0])
    s1 = build_stream(2, lambda s: ((s % npc) >= NPCA).astype(np.int64),
                      [posA, posB])

    return dict(npc=npc, nsub=nsub, nwin=nwin, win=win, npcb=npcb,
                sizeA=sizeA, sizeB=sizeB, streams=[s0, s1], invcnt=invcnt)


# -------------------------------------------------------------- kernel build
def build_kernel(n_nodes, ncores, prep, nb_onehot=8):
    npc, nwin, win = prep["npc"], prep["nwin"], prep["win"]
    npcb, sizeA, sizeB = prep["npcb"], prep["sizeA"], prep["sizeB"]
    streams = prep["streams"]
    spw = win // P

    nc = bacc.Bacc(None, num_swdge_queues=NQ)

    xtab = nc.declare_dram_parameter("xtab", [n_nodes, D], BF16, isOutput=False)
    xT_d = nc.declare_dram_parameter("xT", [D, npc], F32, isOutput=False)
    nc0, nc1 = streams[0]["ncols"], streams[1]["ncols"]
    idx0_d = nc.declare_dram_parameter("idx16_0", [P, nc0 * 8], I16, isOutput=False)
    idx1_d = nc.declare_dram_parameter("idx16_1", [P, nc1 * 8], I16, isOutput=False)
    slots0_d = nc.declare_dram_parameter("slots_0", [P, nc0], BF16, isOutput=False)
    slots1_d = nc.declare_dram_parameter("slots_1", [P, nc1], BF16, isOutput=False)
    invcnt_d = nc.declare_dram_parameter("invcnt", [P, npc], F32, isOutput=False)
    W1l_d = nc.declare_dram_parameter("W1l", [D, D], F32, isOutput=False)
    W1r_d = nc.declare_dram_parameter("W1r", [D, D], F32, isOutput=False)
    W2l_d = nc.declare_dram_parameter("W2l", [D, D], F32, isOutput=False)
    W2r_d = nc.declare_dram_parameter("W2r", [D, D], F32, isOutput=False)
    b1_d = nc.declare_dram_parameter("b1", [D, 1], F32, isOutput=False)
    b2row_d = nc.declare_dram_parameter("b2row", [P, D], F32, isOutput=False)
    iota_d = nc.declare_dram_parameter("iota", [P, P], BF16, isOutput=False)
    ident_d = nc.declare_dram_parameter("ident", [P, P], F32, isOutput=False)
    out_d = nc.declare_dram_parameter("out", [npc, D], F32, isOutput=True)

    from contextlib import ExitStack
    with tile.TileContext(nc) as tc, ExitStack() as es:
        dram = es.enter_context(tc.tile_pool(name="dram", bufs=1, space="DRAM"))
        h_local = dram.tile([npc, D], BF16, tag="hloc")
        h_fullA = dram.tile([sizeA, D], BF16, tag="hfA", addr_space="Shared")
        h_fullB = dram.tile([sizeB, D], BF16, tag="hfB", addr_space="Shared")

        const = es.enter_context(tc.tile_pool(name="const", bufs=1))
        sb = es.enter_context(tc.tile_pool(name="sb", bufs=1))
        msgp = es.enter_context(tc.tile_pool(name="msgp", bufs=12))
        segp = es.enter_context(tc.tile_pool(name="segp", bufs=6))
        aggp = es.enter_context(tc.tile_pool(name="aggp", bufs=3))
        rowp = es.enter_context(tc.tile_pool(name="rowp", bufs=3))
        psA = es.enter_context(tc.tile_pool(name="psA", bufs=2, space="PSUM"))
        psB = es.enter_context(tc.tile_pool(name="psB", bufs=2, space="PSUM"))
        psT = es.enter_context(tc.tile_pool(name="psT", bufs=2, space="PSUM"))

        nc.gpsimd.load_library(mlp_library)

        idx0_sb = const.tile([P, nc0 * 8], I16, tag="idx0")
        idx1_sb = const.tile([P, nc1 * 8], I16, tag="idx1")
        slots0_sb = const.tile([P, nc0], BF16, tag="slots0")
        slots1_sb = const.tile([P, nc1], BF16, tag="slots1")
        invcnt_sb = const.tile([P, npc], F32, tag="invcnt")
        iota_sb = const.tile([P, P], BF16, tag="iota")
        ident_sb = const.tile([P, P], F32, tag="ident")
        W1l_sb = const.tile([D, D], F32, tag="W1l")
        W1r_sb = const.tile([D, D], F32, tag="W1r")
        W2l_sb = const.tile([D, D], F32, tag="W2l")
        W2r_sb = const.tile([D, D], F32, tag="W2r")
        b1_sb = const.tile([D, 1], F32, tag="b1")
        b2row_sb = const.tile([P, D], F32, tag="b2row")
        xT_sb = sb.tile([D, npc], F32, tag="xT")
        hT_sb = sb.tile([D, npc], F32, tag="hT")

        # load order matters: the first gather only needs idx0; the first
        # window's matmuls need slots0/iota/weights/xT; invcnt at first
        # evacuation; layer-2 tables last.
        for t, dd in [(idx0_sb, idx0_d), (slots0_sb, slots0_d),
                      (iota_sb, iota_d), (b1_sb, b1_d),
                      (W1l_sb, W1l_d), (W1r_sb, W1r_d), (xT_sb, xT_d),
                      (invcnt_sb, invcnt_d), (ident_sb, ident_d),
                      (b2row_sb, b2row_d), (W2l_sb, W2l_d), (W2r_sb, W2r_d),
                      (idx1_sb, idx1_d), (slots1_sb, slots1_d)]:
            nc.sync.dma_start(out=t[:], in_=dd[:])

        gq = [0]

        def emit_layer(layer, tabs, idx_sb, slots_sb, stream):
            ncols, blocks = stream["ncols"], stream["blocks"]
            nphases = max(blk["ph"] for blk in blocks) + 1
            ngrp = (ncols + nb_onehot - 1) // nb_onehot
            segs = []
            for g in range(ngrp):
                nbg = min(nb_onehot, ncols - g * nb_onehot)
                seg = segp.tile([P, nb_onehot, P], BF16, tag="seg",
                                name=f"seg{layer}_{g}")
                g0 = g * nb_onehot
                nc.vector.tensor_tensor(
                    out=seg[:, :nbg, :],
                    in0=iota_sb[:, None, :].to_broadcast([P, nbg, P]),
                    in1=slots_sb[:, g0:g0 + nbg, None].to_broadcast([P, nbg, P]),
                    op=mybir.AluOpType.is_equal,
                )
                segs.append(seg)

            # phase-partial aggregation buffer (phases accumulate into SBUF;
            # the last phase combines and emits the window's outputs)
            aggpart = None
            if nphases > 1:
                aggpart = sb.tile([P, npc], F32, tag=f"aggpart{layer}",
                                  name=f"aggpart{layer}")

            for blk in blocks:
                w, ph = blk["w"], blk["ph"]
                n0 = w * win
                wn = min(win, npc - n0)
                nsw = (wn + P - 1) // P
                agg_ps = psA.tile([P, win], F32, tag="aggT",
                                  name=f"agg{layer}_{w}_{ph}")

                # each block is its own PSUM accumulation group (the start
                # flag arms zero-on-first-write for the whole 2KB bank)
                sub_of_b = {}
                col = blk["col0"]
                for t, nbt in blk["subs"]:
                    for bi in range(nbt):
                        sub_of_b[col + bi] = t
                    col += nbt
                blk_first_b = blk["col0"]
                blk_last_b = blk["col0"] + blk["nb"] - 1

                # split the block into near-equal instruction sizes: uniform
                # desc-gen times keep the 4 SWDGE queue pairs in lockstep
                # (a small tail instruction breaks the round-robin rhythm)
                nin = (blk["nb"] + GMAX - 1) // GMAX
                base, extra = divmod(blk["nb"], nin)
                tab = tabs[ph]
                c0 = 0
                for ii in range(nin):
                    cn = base + (1 if ii < extra else 0)
                    msg = msgp.tile([P, GMAX, D], BF16, tag="msg",
                                    name=f"msg{layer}_{w}_{ph}_{ii}")
                    nidx = cn * P
                    b0 = blk["col0"] + c0
                    qn = gq[0]
                    gq[0] = (qn + 1) % NQ
                    nc.gpsimd.dma_gather(
                        out_ap=msg[:, :cn, :],
                        in_ap=tab,
                        idxs_ap=idx_sb[:, b0 * 8:(b0 + cn) * 8],
                        num_idxs=nidx,
                        num_idxs_reg=nidx,
                        elem_size=D,
                        queue_num=qn,
                    )
                    for bi in range(cn):
                        b = b0 + bi
                        t = sub_of_b[b]
                        j = t - w * spw
                        nsl = min(P, npc - t * P)
                        nc.tensor.matmul(
                            out=agg_ps[:, j * P:j * P + nsl],
                            lhsT=msg[:, bi, :],
                            rhs=segs[b // nb_onehot][:, b % nb_onehot, :nsl],
                            start=(b == blk_first_b), stop=(b == blk_last_b),
                        )
                    c0 += cn

                if ph < nphases - 1:
                    # partial phase: stash the window's raw sums in SBUF
                    nc.vector.tensor_copy(out=aggpart[:, n0:n0 + wn],
                                          in_=agg_ps[:, :wn])
                    continue

                aggTs = aggp.tile([P, win], F32, tag="aggTs",
                                  name=f"aggTs{layer}_{w}")
                if nphases > 1:
                    nc.vector.tensor_tensor(
                        out=aggTs[:, :wn], in0=agg_ps[:, :wn],
                        in1=aggpart[:, n0:n0 + wn], op=mybir.AluOpType.add)
                    nc.vector.tensor_tensor(
                        out=aggTs[:, :wn], in0=aggTs[:, :wn],
                        in1=invcnt_sb[:, n0:n0 + wn], op=mybir.AluOpType.mult)
                else:
                    nc.vector.tensor_tensor(
                        out=aggTs[:, :wn], in0=agg_ps[:, :wn],
                        in1=invcnt_sb[:, n0:n0 + wn], op=mybir.AluOpType.mult)

                if layer == 0:
                    ab_ps = psB.tile([P, win], F32, tag="AB", name=f"ab{w}")
                    nc.tensor.matmul(out=ab_ps[:, :wn], lhsT=W1l_sb[:],
                                     rhs=aggTs[:, :wn], start=True, stop=False)
                    nc.tensor.matmul(out=ab_ps[:, :wn], lhsT=W1r_sb[:],
                                     rhs=xT_sb[:, n0:n0 + wn], start=False, stop=True)
                    nc.scalar.activation(
                        out=hT_sb[:, n0:n0 + wn], in_=ab_ps[:, :wn],
                        func=mybir.ActivationFunctionType.Relu,
                        bias=b1_sb[:, 0:1], scale=1.0)
                    for j in range(nsw):
                        r0 = n0 + j * P
                        ns = min(P, npc - r0)
                        tr_ps = psT.tile([P, P], F32, tag="tr", name=f"tr{w}_{j}")
                        nc.tensor.transpose(out=tr_ps[:ns, :],
                                            in_=hT_sb[:, r0:r0 + ns],
                                            identity=ident_sb[:])
                        hrow = rowp.tile([P, D], BF16, tag="hrow",
                                         name=f"hrow{w}_{j}")
                        nc.vector.tensor_copy(out=hrow[:ns, :], in_=tr_ps[:ns, :])
                        nc.sync.dma_start(out=h_local[r0:r0 + ns, :],
                                          in_=hrow[:ns, :])
                    if n0 + wn == NPCA:
                        # chunk A complete: AllGather it now; the transfer
                        # hides under the remaining layer-1 windows.  high
                        # priority pins the trigger right after its deps so
                        # the scheduler cannot push it towards its consumer.
                        with tc.high_priority():
                            nc.gpsimd.collective_compute(
                                "AllGather", mybir.AluOpType.bypass,
                                replica_groups=[list(range(ncores))],
                                ins=[h_local[0:NPCA, :]],
                                outs=[h_fullA[:]])
                    elif n0 + wn == npc:
                        with tc.high_priority():
                            nc.gpsimd.collective_compute(
                                "AllGather", mybir.AluOpType.bypass,
                                replica_groups=[list(range(ncores))],
                                ins=[h_local[NPCA:npc, :]],
                                outs=[h_fullB[:]])
                else:
                    for j in range(nsw):
                        r0 = n0 + j * P
                        ns = min(P, npc - r0)
                        o_ps = psT.tile([P, P], F32, tag="tr", name=f"ops{w}_{j}")
                        nc.tensor.matmul(out=o_ps[:ns, :],
                                         lhsT=aggTs[:, j * P:j * P + ns],
                                         rhs=W2l_sb[:], start=True, stop=False)
                        nc.tensor.matmul(out=o_ps[:ns, :],
                                         lhsT=hT_sb[:, r0:r0 + ns],
                                         rhs=W2r_sb[:], start=False, stop=True)
                        orow = rowp.tile([P, D], F32, tag="orow",
                                         name=f"orow{w}_{j}")
                        nc.vector.tensor_tensor(
                            out=orow[:ns, :], in0=o_ps[:ns, :],
                            in1=b2row_sb[:ns, :], op=mybir.AluOpType.add)
                        nc.sync.dma_start(out=out_d[r0:r0 + ns, :],
                                          in_=orow[:ns, :])

        # phase tables: views based mid-table; signed idx covers the rest.
        # each view overlaps its chunk's written range, so the AllGather ->
        # gather dependency is tracked.
        emit_layer(0, [xtab[32768:n_nodes, :]], idx0_sb, slots0_sb, streams[0])
        emit_layer(1, [h_fullA[sizeA // 2:sizeA, :],
                       h_fullB[sizeB // 2:sizeB, :]],
                   idx1_sb, slots1_sb, streams[1])

    nc.finalize()
    return nc


# ---------------------------------------------------------------- in_maps
def make_in_maps(x, edge_index, W1_l, b1_l, W1_r, W2_l, b2_l, W2_r,
                 n_nodes, ncores, win=512):
    prep = host_prep(edge_index, n_nodes, ncores, win=win)
    npc = prep["npc"]
    x = np.asarray(x, dtype=np.float32)
    xtab = x.astype(ml_dtypes.bfloat16)
    xT = np.ascontiguousarray(x.T)
    iota = np.tile(np.arange(P, dtype=np.float32)[None, :], (P, 1)).astype(
        ml_dtypes.bfloat16)
    ident = np.eye(P, dtype=np.float32)
    common = dict(
        xtab=xtab,
        W1l=np.asarray(W1_l, np.float32), W1r=np.asarray(W1_r, np.float32),
        W2l=np.asarray(W2_l, np.float32), W2r=np.asarray(W2_r, np.float32),
        b1=np.asarray(b1_l, np.float32).reshape(D, 1),
        b2row=np.tile(np.asarray(b2_l, np.float32).reshape(1, D), (P, 1)),
        iota=iota, ident=ident,
    )
    s0, s1 = prep["streams"]
    in_maps = []
    for c in range(ncores):
        in_maps.append(dict(
            common,
            xT=np.ascontiguousarray(xT[:, c * npc:(c + 1) * npc]),
            idx16_0=s0["idx16"][c], idx16_1=s1["idx16"][c],
            slots_0=s0["slots"][c], slots_1=s1["slots"][c],
            invcnt=np.tile(prep["invcnt"][c], (P, 1)),
        ))
    return prep, in_maps


# ------------------------------------------------------------------ kernel()
N_NODES = 50000
NCORES = 8

_cache = {}
last_result = None  # BassKernelResults of the most recent run (for test.py)


def kernel(x, edge_index, W1_l, b1_l, W1_r, W2_l, b2_l, W2_r,
           trace=False, trace_kwargs=None):
    """Full inputs in, full output out. Shards across 8 NeuronCores."""
    global last_result
    from concourse.bass_utils import run_bass_kernel_spmd

    x = np.asarray(x)
    edge_index = np.asarray(edge_index)
    n_nodes = x.shape[0]
    assert n_nodes % NCORES == 0

    prep, in_maps = make_in_maps(x, edge_index, W1_l, b1_l, W1_r,
                                 W2_l, b2_l, W2_r, n_nodes, NCORES)
    key = (n_nodes,
           tuple(blk["nb"] for s in prep["streams"] for blk in s["blocks"]))
    if key not in _cache:
        _cache[key] = build_kernel(n_nodes, NCORES, prep)
    nc = _cache[key]

    res = run_bass_kernel_spmd(nc, in_maps, list(range(NCORES)),
                               trace=trace, **(trace_kwargs or {}))
    last_result = res
    out = np.concatenate([res.results[c]["out"] for c in range(NCORES)],
                         axis=0)
    return out.astype(np.float32)
